# revision 71
# baseline (speedup 1.0000x reference)
"""Trainium2 Bass kernel for the dual channel-attention module.

Data-parallel over batch: B=8 -> one batch item per NeuronCore. Each core runs
two independent pipelines (y -> o1, x -> o2); each pipeline is:
  3x3 conv projections (Q,K stride 2) fused with BatchNorm,
  channel attention S = Q K^T (over tokens), softmax over channels.
The V conv + per-head context + output projection are algebraically folded:
  mean_h P_h @ conv(img, Wv_h) = conv(img, mean_h P_h @ Wv_h)
so after softmax the kernel merges the attention probs into the V-conv
weights on-device (per tap: Wm = sum_h (P_h Wv_h)^T W_out^T / H), then runs a
single stride-1 3x3 conv producing o^T = [C, N] directly; the host
transposes. This cuts the V-path matmul work ~4x.

All matmuls run as float32r (full PE rate at free-dim>=256, fp22 mantissa).
BN scale (and the attention 1/sqrt(C) for Q, and the 1/heads for the output
projection) are folded into weights on the host; Q/K BN bias is applied via a
ones-column bias matmul; the V BN bias is routed through the same prob/W_out
fold into a per-partition output bias.
"""

import os
import sys

sys.path.insert(0, '/opt/trn_rl_repo')

import numpy as np

import concourse.bacc as bacc
import concourse.mybir as mybir
import concourse.tile as tile
from concourse.bass_utils import run_bass_kernel_spmd
from concourse.masks import make_identity

F32 = mybir.dt.float32
F32R = mybir.dt.float32r
BF16 = mybir.dt.bfloat16
AF = mybir.ActivationFunctionType
AX = mybir.AxisListType

P = 128
C = 256          # channels
HEADS = 4
NCORES = 8
EPS = 1e-5

_programs = {}


def _build_program(H, W):
    """One-core program; same NEFF runs SPMD on all 8 cores."""
    N = H * W                 # stride-1 token count
    PH, PW = H + 2, W + 2     # padded image dims
    OH, OW = H // 2, W // 2   # stride-2 output dims
    NQ = OH * OW              # stride-2 token count
    T = NQ // P               # q/k token chunks
    RQ = P // OW              # stride-2 output rows per token chunk
    T2 = N // P               # input token chunks (and proj chunks)
    NT = N // 512             # v-conv tiles of 512 tokens
    RPN = 512 // W            # image rows per v tile
    CC = C // P               # channel chunks (2)

    nc = bacc.Bacc("TRN2", target_bir_lowering=False, debug=False,
                   num_devices=NCORES)

    # ---- I/O ----
    xin = [nc.dram_tensor(f"in{s}", [N, C], F32R, kind="ExternalInput").ap()
           for s in range(2)]
    wqk = nc.dram_tensor("wqk", [2, 2, HEADS // 2, CC, P, 9, 2 * C], BF16,
                         kind="ExternalInput").ap()
    # V weights for the merge, bf16, bias folded in as column 9*C (+ zero pad)
    wvm = nc.dram_tensor("wvm", [2, HEADS, CC, P, 9 * C + 2], BF16,
                         kind="ExternalInput").ap()
    bqk = nc.dram_tensor("bqk", [2, 2, P, HEADS, C], F32R,
                         kind="ExternalInput").ap()
    wo = nc.dram_tensor("wo", [2, CC, P, C], BF16, kind="ExternalInput").ap()
    # outputs are o^T: [C, N]; host transposes back
    outs = [nc.dram_tensor(f"out{s}", [C, N], F32, kind="ExternalOutput").ap()
            for s in range(2)]

    # tap decomposition for stride-2 grids: (dy,dx) -> grid (py,px,b) + row off a
    # grid combos (py, px, b): 6 of them
    combos = [(0, 0, 0), (0, 1, 0), (0, 0, 1), (1, 0, 0), (1, 1, 0), (1, 0, 1)]
    combo_idx = {c: i for i, c in enumerate(combos)}

    with tile.TileContext(nc, pool_alloc_mode="queue") as tc:
        import contextlib
        with contextlib.ExitStack() as est:
            consts = est.enter_context(tc.tile_pool(name="consts", bufs=1))
            sb_work = est.enter_context(tc.tile_pool(name="work", bufs=1))
            ps_tr = est.enter_context(
                tc.tile_pool(name="ps_tr", bufs=4, space="PSUM"))
            ps_ctx = est.enter_context(
                tc.tile_pool(name="ps_ctx", bufs=4, space="PSUM"))

            ident = consts.tile([P, P], F32)
            make_identity(nc, ident[:])
            ones_f = consts.tile([P, P], F32)
            nc.vector.memset(ones_f[:], 1.0)
            ones = consts.tile([P, P], F32R)
            nc.vector.tensor_copy(ones[:], ones_f[:])
            zeros_f = consts.tile([P, 2 * PW], F32)
            nc.vector.memset(zeros_f[:], 0.0)
            ident_b = consts.tile([P, P], BF16)
            nc.vector.tensor_copy(ident_b[:], ident[:])

            def phase_a(s, sb_img):
                """padded channel-major image via PE transposes (bf16)"""
                img = [sb_img.tile([P, PH, PW], BF16, name=f"imgc{s}{cc}",
                                   tag=f"imgc{cc}") for cc in range(CC)]
                for cc in range(CC):
                    # zero borders: top+bottom rows, then left+right cols
                    nc.vector.tensor_copy(
                        img[cc][:, 0:PH:PH - 1, :], zeros_f[:, : 2 * PW]
                        .rearrange("p (a b) -> p a b", a=2))
                    nc.vector.tensor_copy(
                        img[cc][:, 1:PH - 1, 0:PW:PW - 1],
                        zeros_f[:, : 2 * H]
                        .rearrange("p (a b) -> p b a", a=2))
                GB = 4                    # token chunks per batched DMA
                nr = P // W
                for t4 in range(T2 // GB):
                    tok = sb_work.tile([P, GB, C], F32R, name="tok",
                                       tag="tok", bufs=3)
                    nc.sync.dma_start(
                        tok[:],
                        xin[s][t4 * GB * P:(t4 + 1) * GB * P, :]
                        .rearrange("(g p) c -> p g c", p=P))
                    tok_b = sb_work.tile([P, GB, C], BF16, name="tokb",
                                         tag="tokb", bufs=2)
                    for g in range(GB):
                        # per-group cast: transpose g can start after 1/GB
                        # of the conversion
                        nc.vector.tensor_copy(tok_b[:, g, :], tok[:, g, :])
                    for g in range(GB):
                        r0 = ((t4 * GB + g) * P) // W
                        for cc in range(CC):
                            ptp = ps_tr.tile([P, P], BF16, name="ptp",
                                             tag="pst")
                            nc.tensor.transpose(
                                ptp[:], tok_b[:, g, cc * P:(cc + 1) * P],
                                ident_b[:])
                            nc.vector.tensor_copy(
                                img[cc][:, 1 + r0:1 + r0 + nr, 1:1 + W],
                                ptp[:].rearrange("p (a b) -> p a b", a=nr))
                return img

            def phase_b(s, img, sb_gr):
                """parity-compacted grids for stride-2 conv stationary tiles"""
                gr = [[sb_gr.tile([P, (OH + 1) * OW], BF16,
                                  name=f"g{s}{gi}_{cc}", tag=f"g{gi}_{cc}")
                       for cc in range(CC)] for gi in range(6)]
                uh = (OH + 1) // 2
                for gi, (py, px, b) in enumerate(combos):
                    c0 = 2 * b + px
                    for cc in range(CC):
                        for half, (u0, u1) in enumerate([(0, uh),
                                                         (uh, OH + 1)]):
                            dst = gr[gi][cc][:, u0 * OW:u1 * OW] \
                                .rearrange("p (u v) -> p u v", u=u1 - u0)
                            src = img[cc][:, py + 2 * u0: py + 2 * u1 - 1: 2,
                                          c0: c0 + 2 * OW - 1: 2]
                            if (gi + cc + half) % 2:
                                nc.vector.tensor_copy(dst, src)
                            else:
                                nc.scalar.copy(dst, src)
                return gr

            def load_biasb(s, sb_qk):
                biasb = [sb_qk.tile([P, HEADS, C], F32R, name=f"biasb{qk}",
                                    tag=f"biasb{qk}") for qk in range(2)]
                for qk in range(2):
                    nc.sync.dma_start(biasb[qk][:], bqk[s, qk])
                return biasb

            def load_qkw(s, sb_qkw, qk, pr):
                wt = [sb_qkw.tile([P, 9, 2 * C], BF16, name=f"wqk{qk}c{ci}",
                                  tag="qkw", bufs=3) for ci in range(CC)]
                for ci in range(CC):
                    nc.sync.dma_start(wt[ci][:], wqk[s, qk, pr, ci])
                return wt

            def phase_c(s, gr, sb_qkw, sb_qk, pT, biasb, pre_wt=None):
                """Q/K convs (stride 2, token-major) + channel attention.

                Returns deferred closures (dve_part, pe_part) emitting the
                last pair's softmax; the caller sequences them to keep the
                PE transposes off the critical path at the phase boundary."""
                def softmax_dve(pr, s_ps):
                    """reductions/exp/normalize on Vector+Scalar; frees the
                    s_ps PSUM slots. Returns normalized probs tiles."""
                    work_items = [(hl, ccb) for hl in range(2)
                                  for ccb in range(CC)]
                    negmax = {}
                    for hl, ccb in work_items:
                        nm = sb_work.tile([P, 1], F32, name="negmax",
                                          tag=f"negmax{hl}{ccb}")
                        nc.vector.reduce_max(nm[:], s_ps[hl][ccb][:],
                                             axis=AX.X, negate=True)
                        negmax[hl, ccb] = nm
                    e = {}
                    esum = {}
                    for hl, ccb in work_items:
                        ee = sb_work.tile([P, C], F32, name="esm",
                                          tag=f"esm{hl}{ccb}")
                        es = sb_work.tile([P, 1], F32, name="esum",
                                          tag=f"esum{hl}{ccb}")
                        nc.scalar.activation(ee[:], s_ps[hl][ccb][:], AF.Exp,
                                             bias=negmax[hl, ccb][:],
                                             scale=1.0, accum_out=es[:])
                        e[hl, ccb] = ee
                        esum[hl, ccb] = es
                    pn = {}
                    for hl, ccb in work_items:
                        rec = sb_work.tile([P, 1], F32, name="rec",
                                           tag=f"rec{hl}{ccb}")
                        nc.vector.reciprocal(rec[:], esum[hl, ccb][:])
                        pp = sb_work.tile([P, C], BF16, name="pn",
                                          tag=f"pn{hl}{ccb}")
                        nc.vector.tensor_scalar_mul(pp[:], e[hl, ccb][:],
                                                    rec[:])
                        pn[hl, ccb] = pp
                    return pn

                def softmax_tr(pr, pn):
                    """PE transposes of the normalized probs into pT."""
                    for hl, ccb in [(hl, ccb) for hl in range(2)
                                    for ccb in range(CC)]:
                        h = 2 * pr + hl
                        for dc in range(CC):
                            ptp = ps_tr.tile([P, P], BF16, name="ptp2",
                                             tag="pst")
                            nc.tensor.transpose(
                                ptp[:], pn[hl, ccb][:, dc * P:(dc + 1) * P],
                                ident_b[:])
                            nc.vector.tensor_copy(
                                pT[h][:, dc, ccb * P:(ccb + 1) * P],
                                ptp[:])

                def softmax_block(pr, s_ps):
                    softmax_tr(pr, softmax_dve(pr, s_ps))

                deferred = None
                for pr in range(HEADS // 2):
                    s_ps = [[ps_ctx.tile([P, C], F32, name=f"sps{hl}{ccb}",
                                         tag="psc")
                             for ccb in range(CC)] for hl in range(2)]
                    qt_all = [sb_qk.tile([P, 2 * C], BF16, name=f"qt{t}",
                                         tag=f"qt{t}") for t in range(T)]
                    for qk in range(2):
                        if pr == 0 and qk == 0 and pre_wt is not None:
                            wt = pre_wt
                        else:
                            wt = load_qkw(s, sb_qkw, qk, pr)
                        for t in range(T):
                            acc = ps_tr.tile([P, 2 * C], F32, name="qkacc",
                                             tag="pst")
                            first = True
                            for ci in range(CC):
                                for tap in range(9):
                                    dy, dx = tap // 3, tap % 3
                                    gi = combo_idx[(dy & 1, dx & 1, dx >> 1)]
                                    a = dy >> 1
                                    off = (t * RQ + a) * OW
                                    nc.tensor.matmul(
                                        acc[:], gr[gi][ci][:, off:off + P],
                                        wt[ci][:, tap, :],
                                        start=first,
                                        stop=(ci == CC - 1 and tap == 8))
                                    first = False
                            bb = biasb[qk][:, 2 * pr:2 * pr + 2, :] \
                                .rearrange("p a b -> p (a b)")
                            if qk == 0:
                                nc.vector.tensor_add(out=qt_all[t][:],
                                                     in0=acc[:], in1=bb)
                            else:
                                kt = sb_qk.tile([P, 2 * C], BF16, name="kt",
                                                tag="kt", bufs=2)
                                nc.vector.tensor_add(out=kt[:],
                                                     in0=acc[:], in1=bb)
                                for hl in range(2):
                                    for ccb in range(CC):
                                        nc.tensor.matmul(
                                            s_ps[hl][ccb][:],
                                            qt_all[t][:,
                                                      hl * C + ccb * P:
                                                      hl * C + (ccb + 1) * P],
                                            kt[:, hl * C:(hl + 1) * C],
                                            start=(t == 0),
                                            stop=(t == T - 1))
                        if qk == 0 and deferred is not None:
                            # previous pair's softmax+transposes, off the
                            # boundary critical path
                            deferred()
                            deferred = None
                    deferred = (lambda pr=pr, s_ps=s_ps:
                                softmax_block(pr, s_ps))
                # split the last pair's softmax so the caller can emit the
                # DVE part early (freeing the psc ring) and the PE
                # transposes later
                state = {}
                last_pr = HEADS // 2 - 1

                def d_dve(s_ps=s_ps, pr=last_pr):
                    state['pn'] = softmax_dve(pr, s_ps)

                def d_tr(pr=last_pr):
                    softmax_tr(pr, state['pn'])
                return d_dve, d_tr

            def phase_m1(s, pT, sb_mv, deferred):
                """Merge stage 1: U' = sum_h P_h Wv_h per tap (bf16).

                Returns (usb, bvec, wot) for stage 2."""
                wvt = [[sb_mv.tile([P, 9 * C + 2], BF16, name=f"wvm{h}{dc}",
                                   tag=f"wvm{h}{dc}") for dc in range(CC)]
                       for h in range(HEADS)]
                wot = [sb_mv.tile([P, C], BF16, name=f"wo{cq}",
                                  tag=f"wo{cq}") for cq in range(CC)]
                for h in range(HEADS):
                    for dc in range(CC):
                        nc.sync.dma_start(wvt[h][dc][:], wvm[s, h, dc])
                for cq in range(CC):
                    nc.sync.dma_start(wot[cq][:], wo[s, cq])

                hd = [(h, dc) for h in range(HEADS) for dc in range(CC)]
                if deferred is not None:
                    # last head-pair softmax reductions (Vector/Scalar) now:
                    # they free the psc ring slots the stage-1 matmuls below
                    # are about to reuse
                    deferred[0]()
                # stage 1: U'[c, ci] = sum_{h,d} P_h[c,d] Wv_h[d, ci] per tap
                # (with the V BN shift riding along as column 9*C)
                usb = [[sb_mv.tile([P, C], BF16, name=f"usb{cq}{tap}",
                                   tag=f"usb{cq}{tap}") for tap in range(9)]
                       for cq in range(CC)]
                bvec = [sb_mv.tile([P, 1], F32, name=f"bvec{cq}",
                                   tag=f"bvec{cq}") for cq in range(CC)]
                for cq in range(CC):
                    # taps 0-7, two taps packed per PSUM bank
                    u_ps = [ps_ctx.tile([P, 2, C], F32, name=f"ups{j}",
                                        tag="psc") for j in range(4)]
                    for i, (h, dc) in enumerate(hd):
                        if cq == 0 and i == 4 and deferred is not None:
                            # the probs transposes (PE, via the pst ring)
                            # land behind the h=0/1 merge matmuls just issued
                            deferred[1]()
                            deferred = None
                        lhs = pT[h][:, dc, cq * P:(cq + 1) * P]
                        for j in range(4):
                            # one free-512 matmul covers a tap pair (a
                            # single accumulation group per PSUM bank:
                            # start would clear the whole bank)
                            nc.tensor.matmul(
                                u_ps[j][:],
                                lhs, wvt[h][dc][:, 2 * j * C:(2 * j + 2) * C]
                                .rearrange("p (a b) -> p a b", a=2),
                                start=(i == 0), stop=(i == len(hd) - 1))
                    # tap 8 + bias column, separate pass so the pst ring is
                    # free for the deferred softmax transposes above
                    u8 = ps_tr.tile([P, C + 2], F32, name="u8", tag="pst")
                    for i, (h, dc) in enumerate(hd):
                        lhs = pT[h][:, dc, cq * P:(cq + 1) * P]
                        nc.tensor.matmul(u8[:], lhs,
                                         wvt[h][dc][:, 8 * C:9 * C + 2],
                                         start=(i == 0),
                                         stop=(i == len(hd) - 1))
                    for tap in range(8):
                        nc.scalar.copy(usb[cq][tap][:],
                                       u_ps[tap // 2][:, tap % 2, :])
                    nc.scalar.copy(usb[cq][8][:], u8[:, :C])
                    nc.scalar.copy(bvec[cq][:], u8[:, C:C + 1])
                return usb, bvec, wot

            def phase_m2(s, usb, bvec, wot, sb_mv):
                """Merge stage 2: fold W_out; produce the fused conv
                stationary tiles wmsb and the per-partition output bias."""
                # stage 2: Wm[ci, co] = sum_c U'[c, ci] wo[c, co] per tap
                wmsb = [[sb_mv.tile([P, C], BF16, name=f"wm{tap}{ciq}",
                                    tag=f"wm{tap}{ciq}") for ciq in range(CC)]
                        for tap in range(9)]
                for tap in range(9):
                    for ciq in range(CC):
                        wm_ps = ps_tr.tile([P, C], F32, name="wmps",
                                           tag="pst")
                        for cq in range(CC):
                            nc.tensor.matmul(
                                wm_ps[:],
                                usb[cq][tap][:, ciq * P:(ciq + 1) * P],
                                wot[cq][:],
                                start=(cq == 0), stop=(cq == CC - 1))
                        nc.scalar.copy(wmsb[tap][ciq][:], wm_ps[:])
                # output bias: obias[co] = sum_c wo[c, co] bvec[c].
                # The moving operand must not be tiny (ISA check), so
                # broadcast bvec across 128 columns first.
                bvw = [sb_mv.tile([P, P], BF16, name=f"bvw{cq}",
                                  tag=f"bvw{cq}") for cq in range(CC)]
                for cq in range(CC):
                    nc.vector.tensor_scalar_mul(bvw[cq][:], ones_f[:],
                                                bvec[cq][:])
                obias = [sb_mv.tile([P, 1], F32, name=f"obias{coq}",
                                    tag=f"obias{coq}") for coq in range(CC)]
                for coq in range(CC):
                    ob_ps = ps_tr.tile([P, P], F32, name="obps", tag="pst")
                    for cq in range(CC):
                        nc.tensor.matmul(
                            ob_ps[:], wot[cq][:, coq * P:(coq + 1) * P],
                            bvw[cq][:],
                            start=(cq == 0), stop=(cq == CC - 1))
                    nc.scalar.copy(obias[coq][:], ob_ps[:, 0:1])
                return wmsb, obias

            def phase_v(s, img, wmsb, obias, sb_mv):
                """Fused stride-1 output conv: o^T = Wm * img + obias.

                Token tiles are processed in groups of 4 PSUM banks:
                bank interleaving keeps the PE fill/drain pipeline busy
                (consecutive matmuls into one bank serialize), while group
                boundaries stream the output DMA early."""
                GRP = min(NT, 4)
                for coq in range(CC):
                    if s == 1 and coq == CC - 1 and NT >= 4:
                        # finer final groups: the kernel-tail drain+DMA
                        # burst shrinks
                        bounds = list(range(0, NT - GRP, GRP)) \
                            + [NT - GRP, NT - GRP // 2]
                    else:
                        bounds = list(range(0, NT, GRP))
                    for bi, g0 in enumerate(bounds):
                        g1 = bounds[bi + 1] if bi + 1 < len(bounds) else NT
                        nts = range(g0, g1)
                        acc = {nt: (ps_ctx if nt % 2 else ps_tr)
                               .tile([P, 512], F32, name=f"vacc{nt}",
                                     tag=("psc" if nt % 2 else "pst"))
                               for nt in nts}
                        for ciq in range(CC):
                            for tap in range(9):
                                dy, dx = tap // 3, tap % 3
                                lhs = wmsb[tap][ciq][:, coq * P:(coq + 1) * P]
                                for nt in nts:
                                    r0 = nt * RPN
                                    nc.tensor.matmul(
                                        acc[nt][:], lhs,
                                        img[ciq][:, r0 + dy: r0 + dy + RPN,
                                                 dx:dx + W],
                                        start=(ciq == 0 and tap == 0),
                                        stop=(ciq == CC - 1 and tap == 8))
                        for nt in nts:
                            osb = sb_mv.tile([P, 512], F32, name="osb",
                                             tag="osb", bufs=6)
                            nc.scalar.activation(osb[:], acc[nt][:],
                                                 AF.Identity,
                                                 bias=obias[coq][:],
                                                 scale=1.0)
                            nc.sync.dma_start(
                                outs[s][coq * P:(coq + 1) * P,
                                        nt * 512:(nt + 1) * 512], osb[:])

            # ---- interleaved two-stream schedule ----
            st0 = contextlib.ExitStack()
            cst = contextlib.ExitStack()
            sb_gr = cst.enter_context(tc.tile_pool(name="gr0", bufs=1,
                                                   side="right"))
            sb_qkw = cst.enter_context(tc.tile_pool(name="qkw0", bufs=1,
                                                    side="right"))
            sb_qk = cst.enter_context(tc.tile_pool(name="qk0", bufs=1,
                                                   side="right"))
            sb_img0 = st0.enter_context(tc.tile_pool(name="img0", bufs=1))
            sb_keep0 = st0.enter_context(tc.tile_pool(name="keep0", bufs=1))
            img0 = phase_a(0, sb_img0)
            # conv weights + biases queue behind the image token DMAs (the
            # tokens gate the first transposes; these are needed later)
            pre_wt0 = load_qkw(0, sb_qkw, 0, 0)
            biasb0 = load_biasb(0, sb_qk)
            pT0 = [sb_keep0.tile([P, CC, C], BF16, name=f"pT0{h}",
                                 tag=f"pT{h}") for h in range(HEADS)]
            gr0 = phase_b(0, img0, sb_gr)
            defer0 = phase_c(0, gr0, sb_qkw, sb_qk, pT0, biasb0,
                             pre_wt=pre_wt0)
            cst.close()

            sb_img1 = st0.enter_context(tc.tile_pool(name="img1", bufs=1))
            d0 = contextlib.ExitStack()
            sb_mv0 = d0.enter_context(tc.tile_pool(name="mv0", bufs=1))
            usb0, bvec0, wot0 = phase_m1(0, pT0, sb_mv0, defer0)
            # stream-1 image build (own pool: no false dependency on
            # stream-0's img reads) fills PE bubbles around merge stage 2
            img1 = phase_a(1, sb_img1)
            pT1 = [sb_keep0.tile([P, CC, C], BF16, name=f"pT1{h}",
                                 tag=f"pT{h}") for h in range(HEADS)]
            wmsb0, obias0 = phase_m2(0, usb0, bvec0, wot0, sb_mv0)
            phase_v(0, img0, wmsb0, obias0, sb_mv0)
            d0.close()

            # open mv1 BEFORE the stream-1 conv pools: the allocator then
            # places it over the long-dead mv0 zone instead of the
            # just-freed gr1/qkw1 addresses, so the V-merge weight DMAs
            # don't wait for phase_c(1)'s last conv reads (and don't
            # head-of-line-block the DMA dispatch queue behind them)
            dst_ = contextlib.ExitStack()
            sb_mv1 = dst_.enter_context(tc.tile_pool(name="mv1", bufs=1))
            with contextlib.ExitStack() as cst1:
                sb_gr = cst1.enter_context(tc.tile_pool(name="gr1", bufs=1))
                sb_qkw = cst1.enter_context(tc.tile_pool(name="qkw1", bufs=1))
                sb_qk = cst1.enter_context(tc.tile_pool(name="qk1", bufs=1))
                biasb1 = load_biasb(1, sb_qk)
                gr1 = phase_b(1, img1, sb_gr)
                defer1 = phase_c(1, gr1, sb_qkw, sb_qk, pT1, biasb1)
            usb1, bvec1, wot1 = phase_m1(1, pT1, sb_mv1, defer1)
            wmsb1, obias1 = phase_m2(1, usb1, bvec1, wot1, sb_mv1)
            phase_v(1, img1, wmsb1, obias1, sb_mv1)
            dst_.close()
            st0.close()

    nc.compile()
    return nc


def _prep_weights(w_conv, bn_gamma, bn_beta, bn_mean, bn_var, w_out1, w_out2):
    """Fold BN into conv weights/biases and pack into kernel layouts."""
    w_conv = np.asarray(w_conv, np.float32)
    scale = np.asarray(bn_gamma, np.float32) / np.sqrt(
        np.asarray(bn_var, np.float32) + EPS)            # [6,4,256]
    shift = np.asarray(bn_beta, np.float32) - np.asarray(
        bn_mean, np.float32) * scale

    wf = w_conv * scale[:, :, :, None, None, None]       # [6,4,co,ci,3,3]
    sa = 1.0 / np.sqrt(C)
    wf[0] *= sa
    wf[1] *= sa
    shift = shift.copy()
    shift[0] *= sa
    shift[1] *= sa

    # stream s=0 (y->o1): q=conv1, k=conv2, v=conv4
    # stream s=1 (x->o2): q=conv0, k=conv3, v=conv5
    qk_ids = [[1, 2], [0, 3]]
    v_ids = [4, 5]

    import ml_dtypes

    # wqk[s, qk, pair, ci_chunk, ci, tap, (hl,co)] = wf[conv, h, co, ci, dy, dx]
    wqk = np.empty([2, 2, HEADS // 2, C // P, P, 9, 2 * C], np.float32)
    # wvm[s, h, dchunk, d, tap*C + ci] = wf[vconv, h, d, ci, dy, dx]; col 9C
    # carries the V BN shift (bf16 for the merge matmuls)
    wvm = np.zeros([2, HEADS, C // P, P, 9 * C + 2], ml_dtypes.bfloat16)
    for s in range(2):
        for j, conv in enumerate(qk_ids[s]):
            # [pr, hl, co, ci, tap] -> [pr, ci_chunk, ci, tap, hl, co]
            t = wf[conv].reshape(HEADS // 2, 2, C, C, 9).transpose(0, 3, 4, 1, 2)
            wqk[s, j] = t.reshape(HEADS // 2, C // P, P, 9, 2 * C)
        # [h, d, ci, tap] -> [h, d, tap, ci] -> [h, dchunk, d, tap*ci]
        t = wf[v_ids[s]].reshape(HEADS, C, C, 9).transpose(0, 1, 3, 2)
        wvm[s, :, :, :, :9 * C] = t.reshape(HEADS, C // P, P, 9 * C)
        # V BN shift column
        shv = shift[v_ids[s]].reshape(HEADS, C // P, P)
        wvm[s, :, :, :, 9 * C] = shv

    # bqk[s, qk, 128, h, co] = shift[conv][h, co] (replicated across
    # partitions; added on DVE during the PSUM drain)
    bqk = np.empty([2, 2, P, HEADS, C], np.float32)
    for s in range(2):
        for j, conv in enumerate(qk_ids[s]):
            bqk[s, j] = np.broadcast_to(shift[conv][None], (P, HEADS, C))

    # wo[s, cchunk, c, co] = w_out.T / heads
    wo = np.empty([2, C // P, P, C], np.float32)
    wo[0] = (np.asarray(w_out1, np.float32).T / HEADS).reshape(C // P, P, C)
    wo[1] = (np.asarray(w_out2, np.float32).T / HEADS).reshape(C // P, P, C)

    return (wqk.astype(ml_dtypes.bfloat16), wvm, bqk,
            wo.astype(ml_dtypes.bfloat16))


def kernel(x, y, w_conv, bn_gamma, bn_beta, bn_mean, bn_var, w_out1, w_out2,
           h, w):
    H, W = int(h), int(w)
    x = np.asarray(x, np.float32)
    y = np.asarray(y, np.float32)
    B = x.shape[0]
    assert B == NCORES, f"expected B={NCORES}, got {B}"

    key = (H, W)
    if key not in _programs:
        _programs[key] = _build_program(H, W)
    nc = _programs[key]

    wqk, wvm, bqk, wo = _prep_weights(
        w_conv, bn_gamma, bn_beta, bn_mean, bn_var, w_out1, w_out2)

    in_maps = []
    for b in range(B):
        in_maps.append({
            "in0": np.ascontiguousarray(y[b]),   # stream 0: y -> o1
            "in1": np.ascontiguousarray(x[b]),   # stream 1: x -> o2
            "wqk": wqk, "wvm": wvm, "bqk": bqk, "wo": wo,
        })

    trace = bool(int(os.environ.get("KERNEL_TRACE", "0")))
    res = run_bass_kernel_spmd(nc, in_maps, core_ids=list(range(NCORES)),
                               trace=trace)
    if trace:
        tr = res.instructions_and_trace
        print(f"[kernel] HW exec_time_ns={res.exec_time_ns} "
              f"mean={res.mean_exec_time_ns} "
              f"trace={tr[1] if tr else None}")
        kernel.last_exec_ns = res.exec_time_ns
        kernel.last_result = res

    # outputs are o^T [C, N]; transpose back on host
    o1 = np.stack([res.results[b]["out0"].T for b in range(B)])
    o2 = np.stack([res.results[b]["out1"].T for b in range(B)])
    return o1, o2


# revision 75
# speedup vs baseline: 1.0177x; 1.0177x over previous
"""Trainium2 Bass kernel for the dual channel-attention module.

Data-parallel over batch: B=8 -> one batch item per NeuronCore. Each core runs
two independent pipelines (y -> o1, x -> o2); each pipeline is:
  3x3 conv projections (Q,K stride 2) fused with BatchNorm,
  channel attention S = Q K^T (over tokens), softmax over channels.
The V conv + per-head context + output projection are algebraically folded:
  mean_h P_h @ conv(img, Wv_h) = conv(img, mean_h P_h @ Wv_h)
so after softmax the kernel merges the attention probs into the V-conv
weights on-device (per tap: Wm = sum_h (P_h Wv_h)^T W_out^T / H), then runs a
single stride-1 3x3 conv producing o^T = [C, N] directly; the host
transposes. This cuts the V-path matmul work ~4x.

All matmuls run as float32r (full PE rate at free-dim>=256, fp22 mantissa).
BN scale (and the attention 1/sqrt(C) for Q, and the 1/heads for the output
projection) are folded into weights on the host; Q/K BN bias is applied via a
ones-column bias matmul; the V BN bias is routed through the same prob/W_out
fold into a per-partition output bias.
"""

import os
import sys

sys.path.insert(0, '/opt/trn_rl_repo')

import numpy as np

import concourse.bacc as bacc
import concourse.mybir as mybir
import concourse.tile as tile
from concourse.bass_utils import run_bass_kernel_spmd
from concourse.masks import make_identity

F32 = mybir.dt.float32
F32R = mybir.dt.float32r
BF16 = mybir.dt.bfloat16
AF = mybir.ActivationFunctionType
AX = mybir.AxisListType

P = 128
C = 256          # channels
HEADS = 4
NCORES = 8
EPS = 1e-5

_programs = {}


def _build_program(H, W):
    """One-core program; same NEFF runs SPMD on all 8 cores."""
    N = H * W                 # stride-1 token count
    PH, PW = H + 2, W + 2     # padded image dims
    OH, OW = H // 2, W // 2   # stride-2 output dims
    NQ = OH * OW              # stride-2 token count
    T = NQ // P               # q/k token chunks
    RQ = P // OW              # stride-2 output rows per token chunk
    T2 = N // P               # input token chunks (and proj chunks)
    NT = N // 512             # v-conv tiles of 512 tokens
    RPN = 512 // W            # image rows per v tile
    CC = C // P               # channel chunks (2)

    nc = bacc.Bacc("TRN2", target_bir_lowering=False, debug=False,
                   num_devices=NCORES)

    # ---- I/O ----
    xin = [nc.dram_tensor(f"in{s}", [N, C], F32R, kind="ExternalInput").ap()
           for s in range(2)]
    wqk = nc.dram_tensor("wqk", [2, 2, HEADS // 2, CC, P, 9, 2 * C], BF16,
                         kind="ExternalInput").ap()
    # V weights for the merge, bf16, bias folded in as column 9*C (+ zero pad)
    wvm = nc.dram_tensor("wvm", [2, HEADS, CC, P, 9 * C + 2], BF16,
                         kind="ExternalInput").ap()
    bqk = nc.dram_tensor("bqk", [2, 2, P, HEADS, C], F32R,
                         kind="ExternalInput").ap()
    wo = nc.dram_tensor("wo", [2, CC, P, C], BF16, kind="ExternalInput").ap()
    # outputs are o^T: [C, N]; host transposes back
    outs = [nc.dram_tensor(f"out{s}", [C, N], F32, kind="ExternalOutput").ap()
            for s in range(2)]

    # tap decomposition for stride-2 grids: (dy,dx) -> grid (py,px,b) + row off a
    # grid combos (py, px, b): 6 of them
    combos = [(0, 0, 0), (0, 1, 0), (0, 0, 1), (1, 0, 0), (1, 1, 0), (1, 0, 1)]
    combo_idx = {c: i for i, c in enumerate(combos)}

    with tile.TileContext(nc, pool_alloc_mode="queue") as tc:
        import contextlib
        with contextlib.ExitStack() as est:
            consts = est.enter_context(tc.tile_pool(name="consts", bufs=1))
            sb_work = est.enter_context(tc.tile_pool(name="work", bufs=1))
            ps_tr = est.enter_context(
                tc.tile_pool(name="ps_tr", bufs=4, space="PSUM"))
            ps_ctx = est.enter_context(
                tc.tile_pool(name="ps_ctx", bufs=4, space="PSUM"))

            ident = consts.tile([P, P], F32)
            make_identity(nc, ident[:])
            ones_f = consts.tile([P, P], F32)
            nc.vector.memset(ones_f[:], 1.0)
            zeros_f = consts.tile([P, 2 * PW], F32)
            nc.vector.memset(zeros_f[:], 0.0)
            ident_b = consts.tile([P, P], BF16)
            nc.vector.tensor_copy(ident_b[:], ident[:])

            def phase_a(s, sb_img):
                """padded channel-major image via PE transposes (bf16)"""
                img = [sb_img.tile([P, PH, PW], BF16, name=f"imgc{s}{cc}",
                                   tag=f"imgc{cc}") for cc in range(CC)]
                GB = 4                    # token chunks per batched DMA
                nr = P // W
                for t4 in range(T2 // GB):
                    tok = sb_work.tile([P, GB, C], F32R, name="tok",
                                       tag="tok", bufs=3)
                    nc.sync.dma_start(
                        tok[:],
                        xin[s][t4 * GB * P:(t4 + 1) * GB * P, :]
                        .rearrange("(g p) c -> p g c", p=P))
                    tok_b = sb_work.tile([P, GB, C], BF16, name="tokb",
                                         tag="tokb", bufs=2)
                    for g in range(GB):
                        # per-group cast: transpose g can start after 1/GB
                        # of the conversion
                        nc.vector.tensor_copy(tok_b[:, g, :], tok[:, g, :])
                    for g in range(GB):
                        r0 = ((t4 * GB + g) * P) // W
                        for cc in range(CC):
                            ptp = ps_tr.tile([P, P], BF16, name="ptp",
                                             tag="pst")
                            nc.tensor.transpose(
                                ptp[:], tok_b[:, g, cc * P:(cc + 1) * P],
                                ident_b[:])
                            nc.vector.tensor_copy(
                                img[cc][:, 1 + r0:1 + r0 + nr, 1:1 + W],
                                ptp[:].rearrange("p (a b) -> p a b", a=nr))
                # zero borders after the token loop: keeps the DVE queue
                # clear ahead of the first token casts (phase_b's reads
                # are ordered behind these by tile-range deps)
                for cc in range(CC):
                    nc.vector.tensor_copy(
                        img[cc][:, 0:PH:PH - 1, :], zeros_f[:, : 2 * PW]
                        .rearrange("p (a b) -> p a b", a=2))
                    nc.vector.tensor_copy(
                        img[cc][:, 1:PH - 1, 0:PW:PW - 1],
                        zeros_f[:, : 2 * H]
                        .rearrange("p (a b) -> p b a", a=2))
                return img

            def phase_b(s, img, sb_gr):
                """parity-compacted grids for stride-2 conv stationary tiles"""
                gr = [[sb_gr.tile([P, (OH + 1) * OW], BF16,
                                  name=f"g{s}{gi}_{cc}", tag=f"g{gi}_{cc}")
                       for cc in range(CC)] for gi in range(6)]
                uh = (OH + 1) // 2
                for gi, (py, px, b) in enumerate(combos):
                    c0 = 2 * b + px
                    for cc in range(CC):
                        for half, (u0, u1) in enumerate([(0, uh),
                                                         (uh, OH + 1)]):
                            dst = gr[gi][cc][:, u0 * OW:u1 * OW] \
                                .rearrange("p (u v) -> p u v", u=u1 - u0)
                            src = img[cc][:, py + 2 * u0: py + 2 * u1 - 1: 2,
                                          c0: c0 + 2 * OW - 1: 2]
                            if (gi + cc + half) % 2:
                                nc.vector.tensor_copy(dst, src)
                            else:
                                nc.scalar.copy(dst, src)
                return gr

            def load_biasb(s, sb_qk):
                biasb = [sb_qk.tile([P, HEADS, C], F32R, name=f"biasb{qk}",
                                    tag=f"biasb{qk}") for qk in range(2)]
                for qk in range(2):
                    nc.sync.dma_start(biasb[qk][:], bqk[s, qk])
                return biasb

            def load_qkw(s, sb_qkw, qk, pr):
                wt = [sb_qkw.tile([P, 9, 2 * C], BF16, name=f"wqk{qk}c{ci}",
                                  tag="qkw", bufs=3) for ci in range(CC)]
                for ci in range(CC):
                    nc.sync.dma_start(wt[ci][:], wqk[s, qk, pr, ci])
                return wt

            def phase_c(s, gr, sb_qkw, sb_qk, pT, biasb, pre_wt=None):
                """Q/K convs (stride 2, token-major) + channel attention.

                Returns deferred closures (dve_part, pe_part) emitting the
                last pair's softmax; the caller sequences them to keep the
                PE transposes off the critical path at the phase boundary."""
                def softmax_dve(pr, s_ps):
                    """reductions/exp/normalize on Vector+Scalar; frees the
                    s_ps PSUM slots. Returns normalized probs tiles."""
                    work_items = [(hl, ccb) for hl in range(2)
                                  for ccb in range(CC)]
                    negmax = {}
                    for hl, ccb in work_items:
                        nm = sb_work.tile([P, 1], F32, name="negmax",
                                          tag=f"negmax{hl}{ccb}")
                        nc.vector.reduce_max(nm[:], s_ps[hl][ccb][:],
                                             axis=AX.X, negate=True)
                        negmax[hl, ccb] = nm
                    e = {}
                    esum = {}
                    for hl, ccb in work_items:
                        ee = sb_work.tile([P, C], F32, name="esm",
                                          tag=f"esm{hl}{ccb}")
                        es = sb_work.tile([P, 1], F32, name="esum",
                                          tag=f"esum{hl}{ccb}")
                        nc.scalar.activation(ee[:], s_ps[hl][ccb][:], AF.Exp,
                                             bias=negmax[hl, ccb][:],
                                             scale=1.0, accum_out=es[:])
                        e[hl, ccb] = ee
                        esum[hl, ccb] = es
                    pn = {}
                    for hl, ccb in work_items:
                        rec = sb_work.tile([P, 1], F32, name="rec",
                                           tag=f"rec{hl}{ccb}")
                        nc.vector.reciprocal(rec[:], esum[hl, ccb][:])
                        pp = sb_work.tile([P, C], BF16, name="pn",
                                          tag=f"pn{hl}{ccb}")
                        nc.vector.tensor_scalar_mul(pp[:], e[hl, ccb][:],
                                                    rec[:])
                        pn[hl, ccb] = pp
                    return pn

                def softmax_tr(pr, pn):
                    """PE transposes of the normalized probs into pT."""
                    for hl, ccb in [(hl, ccb) for hl in range(2)
                                    for ccb in range(CC)]:
                        h = 2 * pr + hl
                        for dc in range(CC):
                            ptp = ps_tr.tile([P, P], BF16, name="ptp2",
                                             tag="pst")
                            nc.tensor.transpose(
                                ptp[:], pn[hl, ccb][:, dc * P:(dc + 1) * P],
                                ident_b[:])
                            nc.vector.tensor_copy(
                                pT[h][:, dc, ccb * P:(ccb + 1) * P],
                                ptp[:])

                def softmax_block(pr, s_ps):
                    softmax_tr(pr, softmax_dve(pr, s_ps))

                deferred = None
                for pr in range(HEADS // 2):
                    s_ps = [[ps_ctx.tile([P, C], F32, name=f"sps{hl}{ccb}",
                                         tag="psc")
                             for ccb in range(CC)] for hl in range(2)]
                    qt_all = [sb_qk.tile([P, 2 * C], BF16, name=f"qt{t}",
                                         tag=f"qt{t}") for t in range(T)]
                    for qk in range(2):
                        if pr == 0 and qk == 0 and pre_wt is not None:
                            wt = pre_wt
                        else:
                            wt = load_qkw(s, sb_qkw, qk, pr)
                        for t in range(T):
                            acc = ps_tr.tile([P, 2 * C], F32, name="qkacc",
                                             tag="pst")
                            first = True
                            for ci in range(CC):
                                for tap in range(9):
                                    dy, dx = tap // 3, tap % 3
                                    gi = combo_idx[(dy & 1, dx & 1, dx >> 1)]
                                    a = dy >> 1
                                    off = (t * RQ + a) * OW
                                    nc.tensor.matmul(
                                        acc[:], gr[gi][ci][:, off:off + P],
                                        wt[ci][:, tap, :],
                                        start=first,
                                        stop=(ci == CC - 1 and tap == 8))
                                    first = False
                            bb = biasb[qk][:, 2 * pr:2 * pr + 2, :] \
                                .rearrange("p a b -> p (a b)")
                            if qk == 0:
                                nc.vector.tensor_add(out=qt_all[t][:],
                                                     in0=acc[:], in1=bb)
                            else:
                                kt = sb_qk.tile([P, 2 * C], BF16, name="kt",
                                                tag="kt", bufs=2)
                                nc.vector.tensor_add(out=kt[:],
                                                     in0=acc[:], in1=bb)
                                for hl in range(2):
                                    for ccb in range(CC):
                                        nc.tensor.matmul(
                                            s_ps[hl][ccb][:],
                                            qt_all[t][:,
                                                      hl * C + ccb * P:
                                                      hl * C + (ccb + 1) * P],
                                            kt[:, hl * C:(hl + 1) * C],
                                            start=(t == 0),
                                            stop=(t == T - 1))
                        if qk == 0 and deferred is not None:
                            # previous pair's softmax+transposes, off the
                            # boundary critical path
                            deferred()
                            deferred = None
                    deferred = (lambda pr=pr, s_ps=s_ps:
                                softmax_block(pr, s_ps))
                # split the last pair's softmax so the caller can emit the
                # DVE part early (freeing the psc ring) and the PE
                # transposes later
                state = {}
                last_pr = HEADS // 2 - 1

                def d_dve(s_ps=s_ps, pr=last_pr):
                    state['pn'] = softmax_dve(pr, s_ps)

                def d_tr(pr=last_pr):
                    softmax_tr(pr, state['pn'])
                return d_dve, d_tr

            def phase_m1(s, pT, sb_mv, deferred):
                """Merge stage 1: U' = sum_h P_h Wv_h per tap (bf16).

                Returns (usb, bvec, wot) for stage 2."""
                wvt = [[sb_mv.tile([P, 9 * C + 2], BF16, name=f"wvm{h}{dc}",
                                   tag=f"wvm{h}{dc}") for dc in range(CC)]
                       for h in range(HEADS)]
                wot = [sb_mv.tile([P, C], BF16, name=f"wo{cq}",
                                  tag=f"wo{cq}") for cq in range(CC)]
                for h in range(HEADS):
                    for dc in range(CC):
                        nc.sync.dma_start(wvt[h][dc][:], wvm[s, h, dc])
                for cq in range(CC):
                    nc.sync.dma_start(wot[cq][:], wo[s, cq])

                hd = [(h, dc) for h in range(HEADS) for dc in range(CC)]
                if deferred is not None:
                    # last head-pair softmax reductions (Vector/Scalar) now:
                    # they free the psc ring slots the stage-1 matmuls below
                    # are about to reuse
                    deferred[0]()
                # stage 1: U'[c, ci] = sum_{h,d} P_h[c,d] Wv_h[d, ci] per tap
                # (with the V BN shift riding along as column 9*C)
                usb = [[sb_mv.tile([P, C], BF16, name=f"usb{cq}{tap}",
                                   tag=f"usb{cq}{tap}") for tap in range(9)]
                       for cq in range(CC)]
                bvec = [sb_mv.tile([P, 1], F32, name=f"bvec{cq}",
                                   tag=f"bvec{cq}") for cq in range(CC)]
                for cq in range(CC):
                    # taps 0-7, two taps packed per PSUM bank
                    u_ps = [ps_ctx.tile([P, 2, C], F32, name=f"ups{j}",
                                        tag="psc") for j in range(4)]
                    for i, (h, dc) in enumerate(hd):
                        if cq == 0 and i == 4 and deferred is not None:
                            # the probs transposes (PE, via the pst ring)
                            # land behind the h=0/1 merge matmuls just issued
                            deferred[1]()
                            deferred = None
                        lhs = pT[h][:, dc, cq * P:(cq + 1) * P]
                        for j in range(4):
                            # one free-512 matmul covers a tap pair (a
                            # single accumulation group per PSUM bank:
                            # start would clear the whole bank)
                            nc.tensor.matmul(
                                u_ps[j][:],
                                lhs, wvt[h][dc][:, 2 * j * C:(2 * j + 2) * C]
                                .rearrange("p (a b) -> p a b", a=2),
                                start=(i == 0), stop=(i == len(hd) - 1))
                    # tap 8 + bias column, separate pass so the pst ring is
                    # free for the deferred softmax transposes above
                    u8 = ps_tr.tile([P, C + 2], F32, name="u8", tag="pst")
                    for i, (h, dc) in enumerate(hd):
                        lhs = pT[h][:, dc, cq * P:(cq + 1) * P]
                        nc.tensor.matmul(u8[:], lhs,
                                         wvt[h][dc][:, 8 * C:9 * C + 2],
                                         start=(i == 0),
                                         stop=(i == len(hd) - 1))
                    for tap in range(8):
                        nc.scalar.copy(usb[cq][tap][:],
                                       u_ps[tap // 2][:, tap % 2, :])
                    nc.scalar.copy(usb[cq][8][:], u8[:, :C])
                    nc.scalar.copy(bvec[cq][:], u8[:, C:C + 1])
                return usb, bvec, wot

            def phase_m2(s, usb, bvec, wot, sb_mv):
                """Merge stage 2: fold W_out; produce the fused conv
                stationary tiles wmsb and the per-partition output bias."""
                # stage 2: Wm[ci, co] = sum_c U'[c, ci] wo[c, co] per tap
                wmsb = [[sb_mv.tile([P, C], BF16, name=f"wm{tap}{ciq}",
                                    tag=f"wm{tap}{ciq}") for ciq in range(CC)]
                        for tap in range(9)]
                for tap in range(9):
                    for ciq in range(CC):
                        wm_ps = ps_tr.tile([P, C], F32, name="wmps",
                                           tag="pst")
                        for cq in range(CC):
                            nc.tensor.matmul(
                                wm_ps[:],
                                usb[cq][tap][:, ciq * P:(ciq + 1) * P],
                                wot[cq][:],
                                start=(cq == 0), stop=(cq == CC - 1))
                        nc.scalar.copy(wmsb[tap][ciq][:], wm_ps[:])
                # output bias: obias[co] = sum_c wo[c, co] bvec[c].
                # The moving operand must not be tiny (ISA check), so
                # broadcast bvec across 128 columns first.
                bvw = [sb_mv.tile([P, P], BF16, name=f"bvw{cq}",
                                  tag=f"bvw{cq}") for cq in range(CC)]
                for cq in range(CC):
                    nc.vector.tensor_scalar_mul(bvw[cq][:], ones_f[:],
                                                bvec[cq][:])
                obias = [sb_mv.tile([P, 1], F32, name=f"obias{coq}",
                                    tag=f"obias{coq}") for coq in range(CC)]
                for coq in range(CC):
                    ob_ps = ps_tr.tile([P, P], F32, name="obps", tag="pst")
                    for cq in range(CC):
                        nc.tensor.matmul(
                            ob_ps[:], wot[cq][:, coq * P:(coq + 1) * P],
                            bvw[cq][:],
                            start=(cq == 0), stop=(cq == CC - 1))
                    nc.scalar.copy(obias[coq][:], ob_ps[:, 0:1])
                return wmsb, obias

            def phase_v(s, img, wmsb, obias, sb_mv):
                """Fused stride-1 output conv: o^T = Wm * img + obias.

                Token tiles are processed in groups of 4 PSUM banks:
                bank interleaving keeps the PE fill/drain pipeline busy
                (consecutive matmuls into one bank serialize), while group
                boundaries stream the output DMA early."""
                GRP = min(NT, 4)
                for coq in range(CC):
                    if s == 1 and coq == CC - 1 and NT >= 4:
                        # finer final groups: the kernel-tail drain+DMA
                        # burst shrinks
                        bounds = list(range(0, NT - GRP, GRP)) \
                            + [NT - GRP, NT - GRP // 2]
                    else:
                        bounds = list(range(0, NT, GRP))
                    for bi, g0 in enumerate(bounds):
                        g1 = bounds[bi + 1] if bi + 1 < len(bounds) else NT
                        nts = range(g0, g1)
                        acc = {nt: (ps_ctx if nt % 2 else ps_tr)
                               .tile([P, 512], F32, name=f"vacc{nt}",
                                     tag=("psc" if nt % 2 else "pst"))
                               for nt in nts}
                        for ciq in range(CC):
                            for tap in range(9):
                                dy, dx = tap // 3, tap % 3
                                lhs = wmsb[tap][ciq][:, coq * P:(coq + 1) * P]
                                for nt in nts:
                                    r0 = nt * RPN
                                    nc.tensor.matmul(
                                        acc[nt][:], lhs,
                                        img[ciq][:, r0 + dy: r0 + dy + RPN,
                                                 dx:dx + W],
                                        start=(ciq == 0 and tap == 0),
                                        stop=(ciq == CC - 1 and tap == 8))
                        for nt in nts:
                            osb = sb_mv.tile([P, 512], F32, name="osb",
                                             tag="osb", bufs=6)
                            nc.scalar.activation(osb[:], acc[nt][:],
                                                 AF.Identity,
                                                 bias=obias[coq][:],
                                                 scale=1.0)
                            nc.sync.dma_start(
                                outs[s][coq * P:(coq + 1) * P,
                                        nt * 512:(nt + 1) * 512], osb[:])

            # ---- interleaved two-stream schedule ----
            st0 = contextlib.ExitStack()
            cst = contextlib.ExitStack()
            sb_gr = cst.enter_context(tc.tile_pool(name="gr0", bufs=1,
                                                   side="right"))
            sb_qkw = cst.enter_context(tc.tile_pool(name="qkw0", bufs=1,
                                                    side="right"))
            sb_qk = cst.enter_context(tc.tile_pool(name="qk0", bufs=1,
                                                   side="right"))
            sb_img0 = st0.enter_context(tc.tile_pool(name="img0", bufs=1))
            sb_keep0 = st0.enter_context(tc.tile_pool(name="keep0", bufs=1))
            img0 = phase_a(0, sb_img0)
            # conv weights + biases queue behind the image token DMAs (the
            # tokens gate the first transposes; these are needed later)
            pre_wt0 = load_qkw(0, sb_qkw, 0, 0)
            biasb0 = load_biasb(0, sb_qk)
            pT0 = [sb_keep0.tile([P, CC, C], BF16, name=f"pT0{h}",
                                 tag=f"pT{h}") for h in range(HEADS)]
            gr0 = phase_b(0, img0, sb_gr)
            defer0 = phase_c(0, gr0, sb_qkw, sb_qk, pT0, biasb0,
                             pre_wt=pre_wt0)
            cst.close()

            sb_img1 = st0.enter_context(tc.tile_pool(name="img1", bufs=1))
            d0 = contextlib.ExitStack()
            sb_mv0 = d0.enter_context(tc.tile_pool(name="mv0", bufs=1))
            usb0, bvec0, wot0 = phase_m1(0, pT0, sb_mv0, defer0)
            # stream-1 image build (own pool: no false dependency on
            # stream-0's img reads) fills PE bubbles around merge stage 2
            img1 = phase_a(1, sb_img1)
            pT1 = [sb_keep0.tile([P, CC, C], BF16, name=f"pT1{h}",
                                 tag=f"pT{h}") for h in range(HEADS)]
            wmsb0, obias0 = phase_m2(0, usb0, bvec0, wot0, sb_mv0)
            phase_v(0, img0, wmsb0, obias0, sb_mv0)
            d0.close()

            with contextlib.ExitStack() as cst1:
                sb_gr = cst1.enter_context(tc.tile_pool(name="gr1", bufs=1))
                sb_qkw = cst1.enter_context(tc.tile_pool(name="qkw1", bufs=1))
                sb_qk = cst1.enter_context(tc.tile_pool(name="qk1", bufs=1))
                biasb1 = load_biasb(1, sb_qk)
                gr1 = phase_b(1, img1, sb_gr)
                defer1 = phase_c(1, gr1, sb_qkw, sb_qk, pT1, biasb1)
            with contextlib.ExitStack() as dst_:
                sb_mv1 = dst_.enter_context(tc.tile_pool(name="mv1", bufs=1))
                usb1, bvec1, wot1 = phase_m1(1, pT1, sb_mv1, defer1)
                wmsb1, obias1 = phase_m2(1, usb1, bvec1, wot1, sb_mv1)
                phase_v(1, img1, wmsb1, obias1, sb_mv1)
            st0.close()

    nc.compile()
    return nc


def _prep_weights(w_conv, bn_gamma, bn_beta, bn_mean, bn_var, w_out1, w_out2):
    """Fold BN into conv weights/biases and pack into kernel layouts."""
    w_conv = np.asarray(w_conv, np.float32)
    scale = np.asarray(bn_gamma, np.float32) / np.sqrt(
        np.asarray(bn_var, np.float32) + EPS)            # [6,4,256]
    shift = np.asarray(bn_beta, np.float32) - np.asarray(
        bn_mean, np.float32) * scale

    wf = w_conv * scale[:, :, :, None, None, None]       # [6,4,co,ci,3,3]
    sa = 1.0 / np.sqrt(C)
    wf[0] *= sa
    wf[1] *= sa
    shift = shift.copy()
    shift[0] *= sa
    shift[1] *= sa

    # stream s=0 (y->o1): q=conv1, k=conv2, v=conv4
    # stream s=1 (x->o2): q=conv0, k=conv3, v=conv5
    qk_ids = [[1, 2], [0, 3]]
    v_ids = [4, 5]

    import ml_dtypes

    # wqk[s, qk, pair, ci_chunk, ci, tap, (hl,co)] = wf[conv, h, co, ci, dy, dx]
    wqk = np.empty([2, 2, HEADS // 2, C // P, P, 9, 2 * C], np.float32)
    # wvm[s, h, dchunk, d, tap*C + ci] = wf[vconv, h, d, ci, dy, dx]; col 9C
    # carries the V BN shift (bf16 for the merge matmuls)
    wvm = np.zeros([2, HEADS, C // P, P, 9 * C + 2], ml_dtypes.bfloat16)
    for s in range(2):
        for j, conv in enumerate(qk_ids[s]):
            # [pr, hl, co, ci, tap] -> [pr, ci_chunk, ci, tap, hl, co]
            t = wf[conv].reshape(HEADS // 2, 2, C, C, 9).transpose(0, 3, 4, 1, 2)
            wqk[s, j] = t.reshape(HEADS // 2, C // P, P, 9, 2 * C)
        # [h, d, ci, tap] -> [h, d, tap, ci] -> [h, dchunk, d, tap*ci]
        t = wf[v_ids[s]].reshape(HEADS, C, C, 9).transpose(0, 1, 3, 2)
        wvm[s, :, :, :, :9 * C] = t.reshape(HEADS, C // P, P, 9 * C)
        # V BN shift column
        shv = shift[v_ids[s]].reshape(HEADS, C // P, P)
        wvm[s, :, :, :, 9 * C] = shv

    # bqk[s, qk, 128, h, co] = shift[conv][h, co] (replicated across
    # partitions; added on DVE during the PSUM drain)
    bqk = np.empty([2, 2, P, HEADS, C], np.float32)
    for s in range(2):
        for j, conv in enumerate(qk_ids[s]):
            bqk[s, j] = np.broadcast_to(shift[conv][None], (P, HEADS, C))

    # wo[s, cchunk, c, co] = w_out.T / heads
    wo = np.empty([2, C // P, P, C], np.float32)
    wo[0] = (np.asarray(w_out1, np.float32).T / HEADS).reshape(C // P, P, C)
    wo[1] = (np.asarray(w_out2, np.float32).T / HEADS).reshape(C // P, P, C)

    return (wqk.astype(ml_dtypes.bfloat16), wvm, bqk,
            wo.astype(ml_dtypes.bfloat16))


def kernel(x, y, w_conv, bn_gamma, bn_beta, bn_mean, bn_var, w_out1, w_out2,
           h, w):
    H, W = int(h), int(w)
    x = np.asarray(x, np.float32)
    y = np.asarray(y, np.float32)
    B = x.shape[0]
    assert B == NCORES, f"expected B={NCORES}, got {B}"

    key = (H, W)
    if key not in _programs:
        _programs[key] = _build_program(H, W)
    nc = _programs[key]

    wqk, wvm, bqk, wo = _prep_weights(
        w_conv, bn_gamma, bn_beta, bn_mean, bn_var, w_out1, w_out2)

    in_maps = []
    for b in range(B):
        in_maps.append({
            "in0": np.ascontiguousarray(y[b]),   # stream 0: y -> o1
            "in1": np.ascontiguousarray(x[b]),   # stream 1: x -> o2
            "wqk": wqk, "wvm": wvm, "bqk": bqk, "wo": wo,
        })

    trace = bool(int(os.environ.get("KERNEL_TRACE", "0")))
    res = run_bass_kernel_spmd(nc, in_maps, core_ids=list(range(NCORES)),
                               trace=trace)
    if trace:
        tr = res.instructions_and_trace
        print(f"[kernel] HW exec_time_ns={res.exec_time_ns} "
              f"mean={res.mean_exec_time_ns} "
              f"trace={tr[1] if tr else None}")
        kernel.last_exec_ns = res.exec_time_ns
        kernel.last_result = res

    # outputs are o^T [C, N]; transpose back on host
    o1 = np.stack([res.results[b]["out0"].T for b in range(B)])
    o2 = np.stack([res.results[b]["out1"].T for b in range(B)])
    return o1, o2


# revision 77
# speedup vs baseline: 1.0302x; 1.0123x over previous
"""Trainium2 Bass kernel for the dual channel-attention module.

Data-parallel over batch: B=8 -> one batch item per NeuronCore. Each core runs
two independent pipelines (y -> o1, x -> o2); each pipeline is:
  3x3 conv projections (Q,K stride 2) fused with BatchNorm,
  channel attention S = Q K^T (over tokens), softmax over channels.
The V conv + per-head context + output projection are algebraically folded:
  mean_h P_h @ conv(img, Wv_h) = conv(img, mean_h P_h @ Wv_h)
so after softmax the kernel merges the attention probs into the V-conv
weights on-device (per tap: Wm = sum_h (P_h Wv_h)^T W_out^T / H), then runs a
single stride-1 3x3 conv producing o^T = [C, N] directly; the host
transposes. This cuts the V-path matmul work ~4x.

All matmuls run as float32r (full PE rate at free-dim>=256, fp22 mantissa).
BN scale (and the attention 1/sqrt(C) for Q, and the 1/heads for the output
projection) are folded into weights on the host; Q/K BN bias is applied via a
ones-column bias matmul; the V BN bias is routed through the same prob/W_out
fold into a per-partition output bias.
"""

import os
import sys

sys.path.insert(0, '/opt/trn_rl_repo')

import numpy as np

import concourse.bacc as bacc
import concourse.mybir as mybir
import concourse.tile as tile
from concourse.bass_utils import run_bass_kernel_spmd
from concourse.masks import make_identity

F32 = mybir.dt.float32
F32R = mybir.dt.float32r
BF16 = mybir.dt.bfloat16
AF = mybir.ActivationFunctionType
AX = mybir.AxisListType

P = 128
C = 256          # channels
HEADS = 4
NCORES = 8
EPS = 1e-5

_programs = {}


def _build_program(H, W):
    """One-core program; same NEFF runs SPMD on all 8 cores."""
    N = H * W                 # stride-1 token count
    PH, PW = H + 2, W + 2     # padded image dims
    OH, OW = H // 2, W // 2   # stride-2 output dims
    NQ = OH * OW              # stride-2 token count
    T = NQ // P               # q/k token chunks
    RQ = P // OW              # stride-2 output rows per token chunk
    T2 = N // P               # input token chunks (and proj chunks)
    NT = N // 512             # v-conv tiles of 512 tokens
    RPN = 512 // W            # image rows per v tile
    CC = C // P               # channel chunks (2)

    nc = bacc.Bacc("TRN2", target_bir_lowering=False, debug=False,
                   num_devices=NCORES)

    # ---- I/O ----
    xin = [nc.dram_tensor(f"in{s}", [N, C], F32R, kind="ExternalInput").ap()
           for s in range(2)]
    wqk = nc.dram_tensor("wqk", [2, 2, HEADS // 2, CC, P, 9, 2 * C], BF16,
                         kind="ExternalInput").ap()
    # V weights for the merge, bf16, bias folded in as column 9*C (+ zero pad)
    wvm = nc.dram_tensor("wvm", [2, HEADS, CC, P, 9 * C + 2], BF16,
                         kind="ExternalInput").ap()
    bqk = nc.dram_tensor("bqk", [2, 2, P, HEADS, C], F32R,
                         kind="ExternalInput").ap()
    wo = nc.dram_tensor("wo", [2, CC, P, C], BF16, kind="ExternalInput").ap()
    # outputs are o^T: [C, N]; host transposes back
    outs = [nc.dram_tensor(f"out{s}", [C, N], F32, kind="ExternalOutput").ap()
            for s in range(2)]

    # tap decomposition for stride-2 grids: (dy,dx) -> grid (py,px,b) + row off a
    # grid combos (py, px, b): 6 of them
    combos = [(0, 0, 0), (0, 1, 0), (0, 0, 1), (1, 0, 0), (1, 1, 0), (1, 0, 1)]
    combo_idx = {c: i for i, c in enumerate(combos)}

    with tile.TileContext(nc, pool_alloc_mode="queue") as tc:
        import contextlib
        with contextlib.ExitStack() as est:
            consts = est.enter_context(tc.tile_pool(name="consts", bufs=1))
            sb_work = est.enter_context(tc.tile_pool(name="work", bufs=1))
            ps_tr = est.enter_context(
                tc.tile_pool(name="ps_tr", bufs=4, space="PSUM"))
            ps_ctx = est.enter_context(
                tc.tile_pool(name="ps_ctx", bufs=4, space="PSUM"))

            ident = consts.tile([P, P], F32)
            make_identity(nc, ident[:])
            ones_f = consts.tile([P, P], F32)
            nc.vector.memset(ones_f[:], 1.0)
            ones = consts.tile([P, P], F32R)
            nc.vector.tensor_copy(ones[:], ones_f[:])
            zeros_f = consts.tile([P, 2 * PW], F32)
            nc.vector.memset(zeros_f[:], 0.0)
            ident_b = consts.tile([P, P], BF16)
            nc.vector.tensor_copy(ident_b[:], ident[:])

            def phase_a(s, sb_img):
                """padded channel-major image via PE transposes (bf16)"""
                img = [sb_img.tile([P, PH, PW], BF16, name=f"imgc{s}{cc}",
                                   tag=f"imgc{cc}") for cc in range(CC)]
                for cc in range(CC):
                    # zero borders: top+bottom rows, then left+right cols
                    nc.vector.tensor_copy(
                        img[cc][:, 0:PH:PH - 1, :], zeros_f[:, : 2 * PW]
                        .rearrange("p (a b) -> p a b", a=2))
                    nc.vector.tensor_copy(
                        img[cc][:, 1:PH - 1, 0:PW:PW - 1],
                        zeros_f[:, : 2 * H]
                        .rearrange("p (a b) -> p b a", a=2))
                GB = 4                    # token chunks per batched DMA
                nr = P // W
                for t4 in range(T2 // GB):
                    tok = sb_work.tile([P, GB, C], F32R, name="tok",
                                       tag="tok", bufs=3)
                    nc.sync.dma_start(
                        tok[:],
                        xin[s][t4 * GB * P:(t4 + 1) * GB * P, :]
                        .rearrange("(g p) c -> p g c", p=P))
                    tok_b = sb_work.tile([P, GB, C], BF16, name="tokb",
                                         tag="tokb", bufs=2)
                    for g in range(GB):
                        # per-group cast: transpose g can start after 1/GB
                        # of the conversion
                        nc.vector.tensor_copy(tok_b[:, g, :], tok[:, g, :])
                    for g in range(GB):
                        r0 = ((t4 * GB + g) * P) // W
                        for cc in range(CC):
                            ptp = ps_tr.tile([P, P], BF16, name="ptp",
                                             tag="pst")
                            nc.tensor.transpose(
                                ptp[:], tok_b[:, g, cc * P:(cc + 1) * P],
                                ident_b[:])
                            nc.vector.tensor_copy(
                                img[cc][:, 1 + r0:1 + r0 + nr, 1:1 + W],
                                ptp[:].rearrange("p (a b) -> p a b", a=nr))
                return img

            def phase_b(s, img, sb_gr):
                """parity-compacted grids for stride-2 conv stationary tiles"""
                gr = [[sb_gr.tile([P, (OH + 1) * OW], BF16,
                                  name=f"g{s}{gi}_{cc}", tag=f"g{gi}_{cc}")
                       for cc in range(CC)] for gi in range(6)]
                uh = (OH + 1) // 2
                for gi, (py, px, b) in enumerate(combos):
                    c0 = 2 * b + px
                    for cc in range(CC):
                        for half, (u0, u1) in enumerate([(0, uh),
                                                         (uh, OH + 1)]):
                            dst = gr[gi][cc][:, u0 * OW:u1 * OW] \
                                .rearrange("p (u v) -> p u v", u=u1 - u0)
                            src = img[cc][:, py + 2 * u0: py + 2 * u1 - 1: 2,
                                          c0: c0 + 2 * OW - 1: 2]
                            if (gi + cc + half) % 2:
                                nc.vector.tensor_copy(dst, src)
                            else:
                                nc.scalar.copy(dst, src)
                return gr

            def load_biasb(s, sb_qk):
                biasb = [sb_qk.tile([P, HEADS, C], F32R, name=f"biasb{qk}",
                                    tag=f"biasb{qk}") for qk in range(2)]
                for qk in range(2):
                    nc.sync.dma_start(biasb[qk][:], bqk[s, qk])
                return biasb

            def load_qkw(s, sb_qkw, qk, pr):
                wt = [sb_qkw.tile([P, 9, 2 * C], BF16, name=f"wqk{qk}c{ci}",
                                  tag="qkw", bufs=3) for ci in range(CC)]
                for ci in range(CC):
                    nc.sync.dma_start(wt[ci][:], wqk[s, qk, pr, ci])
                return wt

            def phase_c(s, gr, sb_qkw, sb_qk, pT, biasb, pre_wt=None):
                """Q/K convs (stride 2, token-major) + channel attention.

                Returns deferred closures (dve_part, pe_part) emitting the
                last pair's softmax; the caller sequences them to keep the
                PE transposes off the critical path at the phase boundary."""
                def softmax_dve(pr, s_ps):
                    """reductions/exp/normalize on Vector+Scalar; frees the
                    s_ps PSUM slots. Returns normalized probs tiles."""
                    work_items = [(hl, ccb) for hl in range(2)
                                  for ccb in range(CC)]
                    negmax = {}
                    for hl, ccb in work_items:
                        nm = sb_work.tile([P, 1], F32, name="negmax",
                                          tag=f"negmax{hl}{ccb}")
                        nc.vector.reduce_max(nm[:], s_ps[hl][ccb][:],
                                             axis=AX.X, negate=True)
                        negmax[hl, ccb] = nm
                    e = {}
                    esum = {}
                    for hl, ccb in work_items:
                        ee = sb_work.tile([P, C], F32, name="esm",
                                          tag=f"esm{hl}{ccb}")
                        es = sb_work.tile([P, 1], F32, name="esum",
                                          tag=f"esum{hl}{ccb}")
                        nc.scalar.activation(ee[:], s_ps[hl][ccb][:], AF.Exp,
                                             bias=negmax[hl, ccb][:],
                                             scale=1.0, accum_out=es[:])
                        e[hl, ccb] = ee
                        esum[hl, ccb] = es
                    pn = {}
                    for hl, ccb in work_items:
                        rec = sb_work.tile([P, 1], F32, name="rec",
                                           tag=f"rec{hl}{ccb}")
                        nc.vector.reciprocal(rec[:], esum[hl, ccb][:])
                        pp = sb_work.tile([P, C], BF16, name="pn",
                                          tag=f"pn{hl}{ccb}")
                        nc.vector.tensor_scalar_mul(pp[:], e[hl, ccb][:],
                                                    rec[:])
                        pn[hl, ccb] = pp
                    return pn

                def softmax_tr(pr, pn):
                    """PE transposes of the normalized probs into pT."""
                    for hl, ccb in [(hl, ccb) for hl in range(2)
                                    for ccb in range(CC)]:
                        h = 2 * pr + hl
                        for dc in range(CC):
                            ptp = ps_tr.tile([P, P], BF16, name="ptp2",
                                             tag="pst")
                            nc.tensor.transpose(
                                ptp[:], pn[hl, ccb][:, dc * P:(dc + 1) * P],
                                ident_b[:])
                            nc.vector.tensor_copy(
                                pT[h][:, dc, ccb * P:(ccb + 1) * P],
                                ptp[:])

                def softmax_block(pr, s_ps):
                    softmax_tr(pr, softmax_dve(pr, s_ps))

                deferred = None
                for pr in range(HEADS // 2):
                    s_ps = [[ps_ctx.tile([P, C], F32, name=f"sps{hl}{ccb}",
                                         tag="psc")
                             for ccb in range(CC)] for hl in range(2)]
                    qt_all = [sb_qk.tile([P, 2 * C], BF16, name=f"qt{t}",
                                         tag=f"qt{t}") for t in range(T)]
                    for qk in range(2):
                        if pr == 0 and qk == 0 and pre_wt is not None:
                            wt = pre_wt
                        else:
                            wt = load_qkw(s, sb_qkw, qk, pr)
                        for t in range(T):
                            acc = ps_tr.tile([P, 2 * C], F32, name="qkacc",
                                             tag="pst")
                            first = True
                            for ci in range(CC):
                                for tap in range(9):
                                    dy, dx = tap // 3, tap % 3
                                    gi = combo_idx[(dy & 1, dx & 1, dx >> 1)]
                                    a = dy >> 1
                                    off = (t * RQ + a) * OW
                                    nc.tensor.matmul(
                                        acc[:], gr[gi][ci][:, off:off + P],
                                        wt[ci][:, tap, :],
                                        start=first,
                                        stop=(ci == CC - 1 and tap == 8))
                                    first = False
                            bb = biasb[qk][:, 2 * pr:2 * pr + 2, :] \
                                .rearrange("p a b -> p (a b)")
                            if qk == 0:
                                nc.vector.tensor_add(out=qt_all[t][:],
                                                     in0=acc[:], in1=bb)
                            else:
                                kt = sb_qk.tile([P, 2 * C], BF16, name="kt",
                                                tag="kt", bufs=2)
                                nc.vector.tensor_add(out=kt[:],
                                                     in0=acc[:], in1=bb)
                                for hl in range(2):
                                    for ccb in range(CC):
                                        nc.tensor.matmul(
                                            s_ps[hl][ccb][:],
                                            qt_all[t][:,
                                                      hl * C + ccb * P:
                                                      hl * C + (ccb + 1) * P],
                                            kt[:, hl * C:(hl + 1) * C],
                                            start=(t == 0),
                                            stop=(t == T - 1))
                        if qk == 0 and deferred is not None:
                            # previous pair's softmax+transposes, off the
                            # boundary critical path
                            deferred()
                            deferred = None
                    deferred = (lambda pr=pr, s_ps=s_ps:
                                softmax_block(pr, s_ps))
                # split the last pair's softmax so the caller can emit the
                # DVE part early (freeing the psc ring) and the PE
                # transposes later
                state = {}
                last_pr = HEADS // 2 - 1

                def d_dve(s_ps=s_ps, pr=last_pr):
                    state['pn'] = softmax_dve(pr, s_ps)

                def d_tr(pr=last_pr):
                    softmax_tr(pr, state['pn'])
                return d_dve, d_tr

            def phase_m1(s, pT, sb_mv, deferred):
                """Merge stage 1: U' = sum_h P_h Wv_h per tap (bf16).

                Returns (usb, bvec, wot) for stage 2."""
                wvt = [[sb_mv.tile([P, 9 * C + 2], BF16, name=f"wvm{h}{dc}",
                                   tag=f"wvm{h}{dc}") for dc in range(CC)]
                       for h in range(HEADS)]
                wot = [sb_mv.tile([P, C], BF16, name=f"wo{cq}",
                                  tag=f"wo{cq}") for cq in range(CC)]
                for h in range(HEADS):
                    for dc in range(CC):
                        nc.sync.dma_start(wvt[h][dc][:], wvm[s, h, dc])
                for cq in range(CC):
                    nc.sync.dma_start(wot[cq][:], wo[s, cq])

                hd = [(h, dc) for h in range(HEADS) for dc in range(CC)]
                if deferred is not None:
                    # last head-pair softmax reductions (Vector/Scalar) now:
                    # they free the psc ring slots the stage-1 matmuls below
                    # are about to reuse
                    deferred[0]()
                # stage 1: U'[c, ci] = sum_{h,d} P_h[c,d] Wv_h[d, ci] per tap
                # (with the V BN shift riding along as column 9*C)
                usb = [[sb_mv.tile([P, C], BF16, name=f"usb{cq}{tap}",
                                   tag=f"usb{cq}{tap}") for tap in range(9)]
                       for cq in range(CC)]
                bvec = [sb_mv.tile([P, 1], F32, name=f"bvec{cq}",
                                   tag=f"bvec{cq}") for cq in range(CC)]
                for cq in range(CC):
                    # taps 0-7, two taps packed per PSUM bank
                    u_ps = [ps_ctx.tile([P, 2, C], F32, name=f"ups{j}",
                                        tag="psc") for j in range(4)]
                    for i, (h, dc) in enumerate(hd):
                        if cq == 0 and i == 4 and deferred is not None:
                            # the probs transposes (PE, via the pst ring)
                            # land behind the h=0/1 merge matmuls just issued
                            deferred[1]()
                            deferred = None
                        lhs = pT[h][:, dc, cq * P:(cq + 1) * P]
                        for j in range(4):
                            # one free-512 matmul covers a tap pair (a
                            # single accumulation group per PSUM bank:
                            # start would clear the whole bank)
                            nc.tensor.matmul(
                                u_ps[j][:],
                                lhs, wvt[h][dc][:, 2 * j * C:(2 * j + 2) * C]
                                .rearrange("p (a b) -> p a b", a=2),
                                start=(i == 0), stop=(i == len(hd) - 1))
                    # tap 8 + bias column, separate pass so the pst ring is
                    # free for the deferred softmax transposes above
                    u8 = ps_tr.tile([P, C + 2], F32, name="u8", tag="pst")
                    for i, (h, dc) in enumerate(hd):
                        lhs = pT[h][:, dc, cq * P:(cq + 1) * P]
                        nc.tensor.matmul(u8[:], lhs,
                                         wvt[h][dc][:, 8 * C:9 * C + 2],
                                         start=(i == 0),
                                         stop=(i == len(hd) - 1))
                    for tap in range(8):
                        nc.scalar.copy(usb[cq][tap][:],
                                       u_ps[tap // 2][:, tap % 2, :])
                    nc.scalar.copy(usb[cq][8][:], u8[:, :C])
                    nc.scalar.copy(bvec[cq][:], u8[:, C:C + 1])
                return usb, bvec, wot

            def phase_m2(s, usb, bvec, wot, sb_mv):
                """Merge stage 2: fold W_out; produce the fused conv
                stationary tiles wmsb and the per-partition output bias."""
                # stage 2: Wm[ci, co] = sum_c U'[c, ci] wo[c, co] per tap
                wmsb = [[sb_mv.tile([P, C], BF16, name=f"wm{tap}{ciq}",
                                    tag=f"wm{tap}{ciq}") for ciq in range(CC)]
                        for tap in range(9)]
                for tap in range(9):
                    for ciq in range(CC):
                        wm_ps = ps_tr.tile([P, C], F32, name="wmps",
                                           tag="pst")
                        for cq in range(CC):
                            nc.tensor.matmul(
                                wm_ps[:],
                                usb[cq][tap][:, ciq * P:(ciq + 1) * P],
                                wot[cq][:],
                                start=(cq == 0), stop=(cq == CC - 1))
                        nc.scalar.copy(wmsb[tap][ciq][:], wm_ps[:])
                # output bias: obias[co] = sum_c wo[c, co] bvec[c].
                # The moving operand must not be tiny (ISA check), so
                # broadcast bvec across 128 columns first.
                bvw = [sb_mv.tile([P, P], BF16, name=f"bvw{cq}",
                                  tag=f"bvw{cq}") for cq in range(CC)]
                for cq in range(CC):
                    nc.vector.tensor_scalar_mul(bvw[cq][:], ones_f[:],
                                                bvec[cq][:])
                obias = [sb_mv.tile([P, 1], F32, name=f"obias{coq}",
                                    tag=f"obias{coq}") for coq in range(CC)]
                for coq in range(CC):
                    ob_ps = ps_tr.tile([P, P], F32, name="obps", tag="pst")
                    for cq in range(CC):
                        nc.tensor.matmul(
                            ob_ps[:], wot[cq][:, coq * P:(coq + 1) * P],
                            bvw[cq][:],
                            start=(cq == 0), stop=(cq == CC - 1))
                    nc.scalar.copy(obias[coq][:], ob_ps[:, 0:1])
                return wmsb, obias

            def phase_v(s, img, wmsb, obias, sb_mv):
                """Fused stride-1 output conv: o^T = Wm * img + obias.

                Token tiles are processed in groups of 4 PSUM banks:
                bank interleaving keeps the PE fill/drain pipeline busy
                (consecutive matmuls into one bank serialize), while group
                boundaries stream the output DMA early."""
                for coq in range(CC):
                    if s == 1 and coq == CC - 1 and NT >= 4:
                        # finer final groups: the kernel-tail drain+DMA
                        # burst shrinks
                        bounds = [0, NT - 4, NT - 2]
                    else:
                        # full-width group: all PSUM banks interleave,
                        # drains pipeline across the whole pass
                        bounds = [0]
                    for bi, g0 in enumerate(bounds):
                        g1 = bounds[bi + 1] if bi + 1 < len(bounds) else NT
                        nts = range(g0, g1)
                        acc = {nt: (ps_ctx if nt % 2 else ps_tr)
                               .tile([P, 512], F32, name=f"vacc{nt}",
                                     tag=("psc" if nt % 2 else "pst"))
                               for nt in nts}
                        for ciq in range(CC):
                            for tap in range(9):
                                dy, dx = tap // 3, tap % 3
                                lhs = wmsb[tap][ciq][:, coq * P:(coq + 1) * P]
                                for nt in nts:
                                    r0 = nt * RPN
                                    nc.tensor.matmul(
                                        acc[nt][:], lhs,
                                        img[ciq][:, r0 + dy: r0 + dy + RPN,
                                                 dx:dx + W],
                                        start=(ciq == 0 and tap == 0),
                                        stop=(ciq == CC - 1 and tap == 8))
                        for nt in nts:
                            osb = sb_mv.tile([P, 512], F32, name="osb",
                                             tag="osb", bufs=6)
                            nc.scalar.activation(osb[:], acc[nt][:],
                                                 AF.Identity,
                                                 bias=obias[coq][:],
                                                 scale=1.0)
                            nc.sync.dma_start(
                                outs[s][coq * P:(coq + 1) * P,
                                        nt * 512:(nt + 1) * 512], osb[:])

            # ---- interleaved two-stream schedule ----
            st0 = contextlib.ExitStack()
            cst = contextlib.ExitStack()
            sb_gr = cst.enter_context(tc.tile_pool(name="gr0", bufs=1,
                                                   side="right"))
            sb_qkw = cst.enter_context(tc.tile_pool(name="qkw0", bufs=1,
                                                    side="right"))
            sb_qk = cst.enter_context(tc.tile_pool(name="qk0", bufs=1,
                                                   side="right"))
            sb_img0 = st0.enter_context(tc.tile_pool(name="img0", bufs=1))
            sb_keep0 = st0.enter_context(tc.tile_pool(name="keep0", bufs=1))
            img0 = phase_a(0, sb_img0)
            # conv weights + biases queue behind the image token DMAs (the
            # tokens gate the first transposes; these are needed later)
            pre_wt0 = load_qkw(0, sb_qkw, 0, 0)
            biasb0 = load_biasb(0, sb_qk)
            pT0 = [sb_keep0.tile([P, CC, C], BF16, name=f"pT0{h}",
                                 tag=f"pT{h}") for h in range(HEADS)]
            gr0 = phase_b(0, img0, sb_gr)
            defer0 = phase_c(0, gr0, sb_qkw, sb_qk, pT0, biasb0,
                             pre_wt=pre_wt0)
            cst.close()

            sb_img1 = st0.enter_context(tc.tile_pool(name="img1", bufs=1))
            d0 = contextlib.ExitStack()
            sb_mv0 = d0.enter_context(tc.tile_pool(name="mv0", bufs=1))
            usb0, bvec0, wot0 = phase_m1(0, pT0, sb_mv0, defer0)
            # stream-1 image build (own pool: no false dependency on
            # stream-0's img reads) fills PE bubbles around merge stage 2
            img1 = phase_a(1, sb_img1)
            pT1 = [sb_keep0.tile([P, CC, C], BF16, name=f"pT1{h}",
                                 tag=f"pT{h}") for h in range(HEADS)]
            wmsb0, obias0 = phase_m2(0, usb0, bvec0, wot0, sb_mv0)
            phase_v(0, img0, wmsb0, obias0, sb_mv0)
            d0.close()

            with contextlib.ExitStack() as cst1:
                sb_gr = cst1.enter_context(tc.tile_pool(name="gr1", bufs=1))
                sb_qkw = cst1.enter_context(tc.tile_pool(name="qkw1", bufs=1))
                sb_qk = cst1.enter_context(tc.tile_pool(name="qk1", bufs=1))
                biasb1 = load_biasb(1, sb_qk)
                gr1 = phase_b(1, img1, sb_gr)
                defer1 = phase_c(1, gr1, sb_qkw, sb_qk, pT1, biasb1)
            with contextlib.ExitStack() as dst_:
                sb_mv1 = dst_.enter_context(tc.tile_pool(name="mv1", bufs=1))
                usb1, bvec1, wot1 = phase_m1(1, pT1, sb_mv1, defer1)
                wmsb1, obias1 = phase_m2(1, usb1, bvec1, wot1, sb_mv1)
                phase_v(1, img1, wmsb1, obias1, sb_mv1)
            st0.close()

    nc.compile()
    return nc


def _prep_weights(w_conv, bn_gamma, bn_beta, bn_mean, bn_var, w_out1, w_out2):
    """Fold BN into conv weights/biases and pack into kernel layouts."""
    w_conv = np.asarray(w_conv, np.float32)
    scale = np.asarray(bn_gamma, np.float32) / np.sqrt(
        np.asarray(bn_var, np.float32) + EPS)            # [6,4,256]
    shift = np.asarray(bn_beta, np.float32) - np.asarray(
        bn_mean, np.float32) * scale

    wf = w_conv * scale[:, :, :, None, None, None]       # [6,4,co,ci,3,3]
    sa = 1.0 / np.sqrt(C)
    wf[0] *= sa
    wf[1] *= sa
    shift = shift.copy()
    shift[0] *= sa
    shift[1] *= sa

    # stream s=0 (y->o1): q=conv1, k=conv2, v=conv4
    # stream s=1 (x->o2): q=conv0, k=conv3, v=conv5
    qk_ids = [[1, 2], [0, 3]]
    v_ids = [4, 5]

    import ml_dtypes

    # wqk[s, qk, pair, ci_chunk, ci, tap, (hl,co)] = wf[conv, h, co, ci, dy, dx]
    wqk = np.empty([2, 2, HEADS // 2, C // P, P, 9, 2 * C], np.float32)
    # wvm[s, h, dchunk, d, tap*C + ci] = wf[vconv, h, d, ci, dy, dx]; col 9C
    # carries the V BN shift (bf16 for the merge matmuls)
    wvm = np.zeros([2, HEADS, C // P, P, 9 * C + 2], ml_dtypes.bfloat16)
    for s in range(2):
        for j, conv in enumerate(qk_ids[s]):
            # [pr, hl, co, ci, tap] -> [pr, ci_chunk, ci, tap, hl, co]
            t = wf[conv].reshape(HEADS // 2, 2, C, C, 9).transpose(0, 3, 4, 1, 2)
            wqk[s, j] = t.reshape(HEADS // 2, C // P, P, 9, 2 * C)
        # [h, d, ci, tap] -> [h, d, tap, ci] -> [h, dchunk, d, tap*ci]
        t = wf[v_ids[s]].reshape(HEADS, C, C, 9).transpose(0, 1, 3, 2)
        wvm[s, :, :, :, :9 * C] = t.reshape(HEADS, C // P, P, 9 * C)
        # V BN shift column
        shv = shift[v_ids[s]].reshape(HEADS, C // P, P)
        wvm[s, :, :, :, 9 * C] = shv

    # bqk[s, qk, 128, h, co] = shift[conv][h, co] (replicated across
    # partitions; added on DVE during the PSUM drain)
    bqk = np.empty([2, 2, P, HEADS, C], np.float32)
    for s in range(2):
        for j, conv in enumerate(qk_ids[s]):
            bqk[s, j] = np.broadcast_to(shift[conv][None], (P, HEADS, C))

    # wo[s, cchunk, c, co] = w_out.T / heads
    wo = np.empty([2, C // P, P, C], np.float32)
    wo[0] = (np.asarray(w_out1, np.float32).T / HEADS).reshape(C // P, P, C)
    wo[1] = (np.asarray(w_out2, np.float32).T / HEADS).reshape(C // P, P, C)

    return (wqk.astype(ml_dtypes.bfloat16), wvm, bqk,
            wo.astype(ml_dtypes.bfloat16))


def kernel(x, y, w_conv, bn_gamma, bn_beta, bn_mean, bn_var, w_out1, w_out2,
           h, w):
    H, W = int(h), int(w)
    x = np.asarray(x, np.float32)
    y = np.asarray(y, np.float32)
    B = x.shape[0]
    assert B == NCORES, f"expected B={NCORES}, got {B}"

    key = (H, W)
    if key not in _programs:
        _programs[key] = _build_program(H, W)
    nc = _programs[key]

    wqk, wvm, bqk, wo = _prep_weights(
        w_conv, bn_gamma, bn_beta, bn_mean, bn_var, w_out1, w_out2)

    in_maps = []
    for b in range(B):
        in_maps.append({
            "in0": np.ascontiguousarray(y[b]),   # stream 0: y -> o1
            "in1": np.ascontiguousarray(x[b]),   # stream 1: x -> o2
            "wqk": wqk, "wvm": wvm, "bqk": bqk, "wo": wo,
        })

    trace = bool(int(os.environ.get("KERNEL_TRACE", "0")))
    res = run_bass_kernel_spmd(nc, in_maps, core_ids=list(range(NCORES)),
                               trace=trace)
    if trace:
        tr = res.instructions_and_trace
        print(f"[kernel] HW exec_time_ns={res.exec_time_ns} "
              f"mean={res.mean_exec_time_ns} "
              f"trace={tr[1] if tr else None}")
        kernel.last_exec_ns = res.exec_time_ns
        kernel.last_result = res

    # outputs are o^T [C, N]; transpose back on host
    o1 = np.stack([res.results[b]["out0"].T for b in range(B)])
    o2 = np.stack([res.results[b]["out1"].T for b in range(B)])
    return o1, o2
